# revision 16
# baseline (speedup 1.0000x reference)
"""Trainium2 Bass kernel for nn_MultiHeadSelfTokenAttention.

Reference computation (per (b, s) slice, X = hidden[b, s] in [T=128, H=768]):
    q      = X @ Wq + bq                       [T, 12]     (per-token per-head logit)
    scores = q + mask[:, None] * (-10000)
    alpha  = softmax(scores, axis=T)           [T, 12]
    v      = (X @ Wv + bv).reshape(T, 12, 64)
    res    = einsum('th,thd->hd', alpha, v)    [12, 64] -> [768]
    out    = LN(res @ Wo + bo) * gamma + beta  [768]

Algebraic restructure: the pooled value P = sum_t alpha * V is computed as
    Y[head, h] = sum_t e[t, head] * X[t, h]          (UNNORMALIZED exp weights)
    P[head, :] = (Y[head, :] @ Wv[:, head*64:...]) * zinv[head]
with the softmax normalization deferred to the tiny pooled tensor P, and
bv folded into the output bias on the host (bo2 = bo + bv @ Wo).  V is
never materialized.

bf16 datapath (halves HBM traffic, 1-cycle/row PE streaming, FWL weight
loads).  X -> X^T runs on the PE (identity transposes); the small
exp(q)^T -> exp(q) transposes run on the DMA XBAR transpose unit (one
instruction per 8-sent block).  Softmax sums and layernorm statistics stay
in f32.

Sharding: data-parallel across batch; core b handles hidden_states[b]
(32 sents).  Weights replicated.  No collectives.
"""

import os
import sys
from contextlib import ExitStack

import numpy as np
import ml_dtypes

for _p in ("/opt/trn_rl_repo", "/root/.axon_site/_ro/trn_rl_repo"):
    if os.path.isdir(_p) and _p not in sys.path:
        sys.path.insert(0, _p)

import concourse.bacc as bacc
import concourse.bass as bass
import concourse.tile as tile
from concourse import mybir
from concourse.bass_utils import run_bass_kernel_spmd

F32 = mybir.dt.float32
BF16 = mybir.dt.bfloat16
AF = mybir.ActivationFunctionType
ALU = mybir.AluOpType

HIDDEN = 768
HEADS = 12
B, S, T = 8, 32, 128
HC = HIDDEN // 128  # 6 chunks of the hidden dim
LN_EPS = 1e-5
MASK_NEG = -10000.0
N_CORES = 8
BS = 8            # sents per block
NBLK = S // BS    # 4 blocks


def build_kernel():
    nc = bacc.Bacc(trn_type="TRN2", target_bir_lowering=False, debug=False)

    hs = nc.dram_tensor("hs", [S, T, HIDDEN], BF16, kind="ExternalInput").ap()
    mask = nc.dram_tensor("mask", [S * T], BF16, kind="ExternalInput").ap()
    # host-side pre-rearranged weights
    wq = nc.dram_tensor("wq", [128, HC * HEADS], BF16, kind="ExternalInput").ap()
    bq = nc.dram_tensor("bq", [HEADS], BF16, kind="ExternalInput").ap()
    wv = nc.dram_tensor("wv", [128, HC * HIDDEN], BF16, kind="ExternalInput").ap()
    wo = nc.dram_tensor("wo", [128, HC * HIDDEN], BF16, kind="ExternalInput").ap()
    bo2 = nc.dram_tensor("bo2", [HIDDEN], BF16, kind="ExternalInput").ap()
    sel = nc.dram_tensor("sel", [HEADS, HC * 128], BF16, kind="ExternalInput").ap()
    grep = nc.dram_tensor("grep", [S, HIDDEN], F32, kind="ExternalInput").ap()
    brep = nc.dram_tensor("brep", [S, HIDDEN], F32, kind="ExternalInput").ap()
    ident = nc.dram_tensor("ident", [128, 128], BF16, kind="ExternalInput").ap()
    out = nc.dram_tensor("out", [S, HIDDEN], F32, kind="ExternalOutput").ap()

    with tile.TileContext(nc) as tc:
        kernel_body(
            tc, out, hs, mask, wq, bq, wv, wo, bo2, sel, grep, brep, ident
        )
    nc.compile()
    return nc


def kernel_body(tc, out, hs, mask, wq, bq, wv, wo, bo2, sel, grep, brep, ident):
    nc = tc.nc
    with ExitStack() as ctx:
        consts = ctx.enter_context(tc.tile_pool(name="consts", bufs=1))
        xp = ctx.enter_context(tc.tile_pool(name="x", bufs=3))
        xtp = ctx.enter_context(tc.tile_pool(name="xt", bufs=2))
        smallp = ctx.enter_context(tc.tile_pool(name="small", bufs=2))
        ps_yt = ctx.enter_context(tc.tile_pool(name="ps_yt", bufs=2, space="PSUM"))
        ps_aq = ExitStack()
        ps_xt = ps_aq.enter_context(tc.tile_pool(name="ps_xt", bufs=2, space="PSUM"))
        ps_qt = ps_aq.enter_context(tc.tile_pool(name="ps_qt", bufs=2, space="PSUM"))

        # --------- tiles for constants (DMAs are emitted inside the schedule
        # so the rings drain in consumption order) ---------------------------
        ident_sb = consts.tile([128, 128], BF16, tag="ident")
        wq_sb = consts.tile([128, HC * HEADS], BF16, tag="wq")
        extras_w = consts.tile([2, HEADS], BF16, tag="exw")
        extras_rhs = consts.tile([2, S * T], BF16, tag="exr")
        ones_col = consts.tile([1, S], BF16, tag="ones")
        bo_row = consts.tile([1, HIDDEN], BF16, tag="bo")
        sel_sb = consts.tile([HEADS, HC * 128], BF16, tag="sel")
        wv_sb = consts.tile([128, HC * HIDDEN], BF16, tag="wv")
        wo_sb = consts.tile([128, HC * HIDDEN], BF16, tag="wo")
        gamma_rep = consts.tile([S, HIDDEN], F32, tag="grep")
        beta_rep = consts.tile([S, HIDDEN], F32, tag="brep")
        # Y^T accumulator (unnormalized): yt_all[p, c*S*12 + s*12 + n]
        yt_all = consts.tile([128, HC * S * HEADS], BF16, tag="yt", name="yt_all")
        # per-(head, sent) softmax normalizers
        zinv_all = consts.tile([HEADS, S], F32, tag="zinv", name="zinv_all")

        # identity first on the scalar ring: transposes need it immediately
        nc.scalar.dma_start(ident_sb[:], ident[:])

        # warm the ACT Exp table during the initial DMA wait
        warm = consts.tile([1, 2], F32, tag="warm", name="warm")
        nc.vector.memset(warm[:], 1.0)
        warm2 = consts.tile([1, 2], F32, tag="warm2", name="warm2")
        nc.scalar.activation(warm2[:, 1:2], warm[:, 1:2], AF.Exp)

        nc.vector.memset(extras_w[0:1, :], MASK_NEG)
        nc.vector.memset(extras_rhs[:], 1.0)  # row 1 stays all-ones
        nc.vector.memset(ones_col[:], 1.0)

        # ---------------- pipeline stages ------------------------------------
        def stage_a(blk):
            """DMA one block of X, PE-transpose it into X^T."""
            s0 = blk * BS
            x_blk = xp.tile([128, BS * HIDDEN], BF16, tag="xblk", name="x_blk")
            if blk == 0:
                # split the first block across both rings to halve startup
                for sp in range(BS):
                    eng = nc.sync if sp % 2 == 0 else nc.scalar
                    eng.dma_start(
                        x_blk[:, sp * HIDDEN : (sp + 1) * HIDDEN], hs[s0 + sp]
                    )
            else:
                nc.sync.dma_start(
                    x_blk[:], hs[s0 : s0 + BS].rearrange("s t h -> t s h")
                )

            # X^T block in SBUF: col = s'*768 + c*128 + t  (PE transposes)
            xt_blk = xtp.tile([128, BS * HIDDEN], BF16, tag="xtblk", name="xt_blk")
            for sp in range(BS):
                xt_ps = ps_xt.tile([128, HIDDEN], BF16, tag="xtps", name="xt_ps")
                for c in range(HC):
                    nc.tensor.transpose(
                        xt_ps[:, c * 128 : (c + 1) * 128],
                        x_blk[
                            :, sp * HIDDEN + c * 128 : sp * HIDDEN + (c + 1) * 128
                        ],
                        ident_sb[:],
                    )
                eng = nc.vector.tensor_copy if sp % 2 == 0 else nc.scalar.copy
                eng(xt_blk[:, sp * HIDDEN : (sp + 1) * HIDDEN], xt_ps[:])
            return x_blk, xt_blk

        def stage_q(blk, xt_blk):
            """q^T logits + exp (UNNORMALIZED) + row sums for deferred norm."""
            s0 = blk * BS
            qt_ps = ps_qt.tile([HEADS, BS * T], F32, tag="qt", name="qt_ps")
            xt_r = xt_blk.rearrange("p (s c j) -> p c s j", s=BS, j=128)
            spw = 512 // T  # sents per 512-col matmul
            nh = BS // spw
            for h in range(nh):
                for c in range(HC):
                    nc.tensor.matmul(
                        qt_ps[:, h * 512 : (h + 1) * 512],
                        wq_sb[:, c * HEADS : (c + 1) * HEADS],
                        xt_r[:, c, h * spw : (h + 1) * spw],
                        start=(c == 0),
                        stop=False,
                    )
                nc.tensor.matmul(
                    qt_ps[:, h * 512 : (h + 1) * 512],
                    extras_w[:],
                    extras_rhs[:, s0 * T + h * 512 : s0 * T + (h + 1) * 512],
                    start=False,
                    stop=True,
                )

            # e = exp(scores), bf16, padded to 16 partitions for the XBAR
            # (no max-subtraction: unmasked logits are O(5); masked logits
            # are ~-1e4 and exp underflows to exactly 0)
            et_sb = smallp.tile([16, BS * T], BF16, tag="et", name="et_sb")
            nc.vector.memset(et_sb[:], 0.0)
            for h in range(nh):
                nc.scalar.activation(
                    et_sb[0:HEADS, h * 512 : (h + 1) * 512],
                    qt_ps[:, h * 512 : (h + 1) * 512],
                    AF.Exp,
                )
            # row sums -> zinv (consumed only by the endgame)
            zsum = smallp.tile([HEADS, BS], F32, tag="zsum", name="zsum")
            for sp in range(BS):
                nc.vector.tensor_reduce(
                    zsum[:, sp : sp + 1],
                    et_sb[0:HEADS, sp * T : (sp + 1) * T],
                    axis=mybir.AxisListType.X,
                    op=ALU.add,
                )
            nc.vector.reciprocal(zinv_all[:, s0 : s0 + BS], zsum[:])
            return et_sb

        def stage_ty(blk, x_blk, et_sb):
            """XBAR e^T -> e, then Y^T = X^T @ e on the PE (unnormalized)."""
            e_sb = smallp.tile([128, BS * 16], BF16, tag="e", name="e_sb")
            nc.scalar.dma_start(
                e_sb.rearrange("p (s n) -> p s n", n=16), et_sb[:], transpose=True
            )
            for hb in range(2):
                yt_ps = ps_yt.tile(
                    [128, HC * 4 * HEADS], F32, tag="ytps", name="yt_ps"
                )
                for hp in range(4):
                    sp = hb * 4 + hp
                    for c in range(HC):
                        nc.tensor.matmul(
                            yt_ps[
                                :,
                                c * 4 * HEADS
                                + hp * HEADS : c * 4 * HEADS
                                + (hp + 1) * HEADS,
                            ],
                            x_blk[
                                :,
                                sp * HIDDEN + c * 128 : sp * HIDDEN + (c + 1) * 128,
                            ],
                            e_sb[:, sp * 16 : sp * 16 + HEADS],
                        )
                off = (blk * BS + hb * 4) * HEADS
                dst = yt_all.rearrange("p (c k) -> p c k", c=HC)[
                    :, :, off : off + 4 * HEADS
                ]
                src = yt_ps.rearrange("p (c k) -> p c k", c=HC)
                eng = nc.vector.tensor_copy if hb == 0 else nc.scalar.copy
                eng(dst, src)

        # ---------------- schedule -------------------------------------------
        a0 = stage_a(0)
        # small consts follow block 0's scalar-ring half
        nc.scalar.dma_start(wq_sb[:], wq[:])
        nc.scalar.dma_start(extras_w[1:2, :], bq[None, :])
        nc.scalar.dma_start(extras_rhs[0:1, :], mask[None, :])
        nc.scalar.dma_start(bo_row[:], bo2[None, :])
        nc.scalar.dma_start(sel_sb[:], sel[:])
        at0 = stage_q(0, a0[1])
        a1 = stage_a(1)
        at1 = stage_q(1, a1[1])
        stage_ty(0, a0[0], at0)
        a2 = stage_a(2)
        at2 = stage_q(2, a2[1])
        stage_ty(1, a1[0], at1)
        a3 = stage_a(3)
        at3 = stage_q(3, a3[1])
        # warm the Square/Sqrt ACT tables while the PE chews on Y/G
        nc.scalar.activation(warm2[:, 0:1], warm[:, 0:1], AF.Square)
        nc.scalar.activation(warm2[:, 0:1], warm[:, 0:1], AF.Sqrt)
        # big weights on the sync ring, behind the four X block loads
        nc.sync.dma_start(wv_sb[:], wv[:])
        nc.sync.dma_start(wo_sb[:], wo[:])
        nc.sync.dma_start(gamma_rep[:], grep[:])
        nc.sync.dma_start(beta_rep[:], brep[:])
        stage_ty(2, a2[0], at2)
        stage_ty(3, a3[0], at3)
        ps_aq.close()  # free xt/qt PSUM banks for the endgame pools

        # ---------------- endgame: G-route + out-proj + layernorm ------------
        with (
            tc.tile_pool(name="ps_g", bufs=2, space="PSUM") as ps_g,
            tc.tile_pool(name="ps_o", bufs=1, space="PSUM") as ps_o,
        ):
            # zinv replicated to the d-partition layout:
            # zrep[p, dc*S + s] = zinv[2*dc + p//64, s]
            zinv_bf = consts.tile([HEADS, S], BF16, tag="zbf", name="zinv_bf")
            nc.vector.tensor_copy(zinv_bf[:], zinv_all[:])
            zrep_ps = ps_o.tile([128, HC * S], F32, tag="zrep", name="zrep_ps")
            for dc in range(HC):
                nc.tensor.matmul(
                    zrep_ps[:, dc * S : (dc + 1) * S],
                    sel_sb[:, dc * 128 : (dc + 1) * 128],
                    zinv_bf[:],
                )
            # tensor_tensor may read only one PSUM operand -> stage in SBUF
            zrep_sb = consts.tile([128, HC * S], F32, tag="zrs", name="zrep_sb")
            nc.scalar.copy(zrep_sb[:], zrep_ps[:])

            pt_sb = consts.tile([128, HC * S], BF16, tag="pt", name="pt_sb")
            for dc in range(HC):
                g_ps = ps_g.tile([128, S * HEADS], F32, tag="g", name="g_ps")
                for c in range(HC):
                    nc.tensor.matmul(
                        g_ps[:],
                        wv_sb[
                            :, c * HIDDEN + dc * 128 : c * HIDDEN + (dc + 1) * 128
                        ],
                        yt_all[:, c * S * HEADS : (c + 1) * S * HEADS],
                        start=(c == 0),
                        stop=(c == HC - 1),
                    )
                g_r = g_ps.rearrange("p (s n) -> p s n", n=HEADS)
                for hh in range(2):
                    head = 2 * dc + hh
                    nc.vector.tensor_tensor(
                        pt_sb[hh * 64 : hh * 64 + 64, dc * S : (dc + 1) * S],
                        g_r[hh * 64 : hh * 64 + 64, :, head],
                        zrep_sb[hh * 64 : hh * 64 + 64, dc * S : (dc + 1) * S],
                        op=ALU.mult,
                    )

            # out = P @ Wo + bo2   -> [32, 768]
            o1 = ps_o.tile([S, 512], F32, tag="o1", name="o1")
            o2 = ps_o.tile([S, 256], F32, tag="o2", name="o2")
            for dc in range(HC):
                nc.tensor.matmul(
                    o1[:],
                    pt_sb[:, dc * S : (dc + 1) * S],
                    wo_sb[:, dc * HIDDEN : dc * HIDDEN + 512],
                    start=(dc == 0),
                    stop=False,
                )
                nc.tensor.matmul(
                    o2[:],
                    pt_sb[:, dc * S : (dc + 1) * S],
                    wo_sb[:, dc * HIDDEN + 512 : (dc + 1) * HIDDEN],
                    start=(dc == 0),
                    stop=False,
                )
            nc.tensor.matmul(
                o1[:], ones_col[:], bo_row[:, 0:512], start=False, stop=True
            )
            nc.tensor.matmul(
                o2[:], ones_col[:], bo_row[:, 512:768], start=False, stop=True
            )

            # ---------------- layernorm ------------------------------------
            res_sb = consts.tile([S, HIDDEN], F32, tag="res", name="res_sb")
            mu_parts = consts.tile([S, 2], F32, tag="mup", name="mu_parts")
            nc.scalar.activation(
                res_sb[:, 0:512], o1[:], AF.Copy, accum_out=mu_parts[:, 0:1]
            )
            nc.scalar.activation(
                res_sb[:, 512:768], o2[:], AF.Copy, accum_out=mu_parts[:, 1:2]
            )
            mu = consts.tile([S, 1], F32, tag="mu", name="mu")
            nc.vector.tensor_reduce(
                mu[:], mu_parts[:], axis=mybir.AxisListType.X, op=ALU.add
            )
            muv = consts.tile([S, 1], F32, tag="muv", name="muv")
            nc.vector.tensor_scalar_mul(muv[:], mu[:], 1.0 / HIDDEN)
            xc = consts.tile([S, HIDDEN], F32, tag="xc", name="xc")
            nc.vector.tensor_scalar_sub(xc[:], res_sb[:], muv[:])
            sq = consts.tile([S, HIDDEN], F32, tag="sq", name="sq")
            varsum = consts.tile([S, 1], F32, tag="vs", name="varsum")
            nc.scalar.activation(sq[:], xc[:], AF.Square, accum_out=varsum[:])
            vareps = consts.tile([S, 1], F32, tag="ve", name="vareps")
            nc.vector.tensor_scalar(
                vareps[:], varsum[:], 1.0 / HIDDEN, LN_EPS, op0=ALU.mult, op1=ALU.add
            )
            sd = consts.tile([S, 1], F32, tag="sd", name="sd")
            nc.scalar.activation(sd[:], vareps[:], AF.Sqrt)
            rstd = consts.tile([S, 1], F32, tag="rstd", name="rstd")
            nc.vector.reciprocal(rstd[:], sd[:])
            t1 = consts.tile([S, HIDDEN], F32, tag="t1", name="t1")
            nc.vector.scalar_tensor_tensor(
                t1[:], xc[:], rstd[:], gamma_rep[:], op0=ALU.mult, op1=ALU.mult
            )
            out_sb = consts.tile([S, HIDDEN], F32, tag="osb", name="out_sb")
            nc.vector.tensor_add(out_sb[:], t1[:], beta_rep[:])
            nc.sync.dma_start(out[:], out_sb[:])


_NC_CACHE = {}


def kernel(hidden_states, mask, Wq, bq, Wv, bv, Wo, bo, gamma, beta):
    if "nc" not in _NC_CACHE:
        _NC_CACHE["nc"] = build_kernel()
    nc = _NC_CACHE["nc"]
    bf = ml_dtypes.bfloat16
    f32 = np.float32

    Wq_r = np.ascontiguousarray(
        np.asarray(Wq, dtype=f32).reshape(HC, 128, HEADS).transpose(1, 0, 2)
        .reshape(128, HC * HEADS).astype(bf)
    )
    Wv_r = np.ascontiguousarray(
        np.asarray(Wv, dtype=f32).reshape(HC, 128, HIDDEN).transpose(1, 0, 2)
        .reshape(128, HC * HIDDEN).astype(bf)
    )
    Wo_r = np.ascontiguousarray(
        np.asarray(Wo, dtype=f32).reshape(HC, 128, HIDDEN).transpose(1, 0, 2)
        .reshape(128, HC * HIDDEN).astype(bf)
    )
    bq_b = np.asarray(bq, dtype=f32).astype(bf)
    # fold bv through the output projection: (P0 + bv) @ Wo + bo
    bo2 = (
        np.asarray(bo, dtype=f32)
        + np.asarray(bv, dtype=f32) @ np.asarray(Wo, dtype=f32)
    ).astype(bf)
    sel = np.zeros((HEADS, HC * 128), dtype=f32)
    for dc in range(HC):
        for p in range(128):
            sel[2 * dc + p // 64, dc * 128 + p] = 1.0
    sel_b = sel.astype(bf)
    grep = np.ascontiguousarray(np.tile(np.asarray(gamma, dtype=f32), (S, 1)))
    brep = np.ascontiguousarray(np.tile(np.asarray(beta, dtype=f32), (S, 1)))
    ident_b = np.eye(128, dtype=f32).astype(bf)

    in_maps = [
        {
            "hs": np.ascontiguousarray(np.asarray(hidden_states[b], dtype=f32)).astype(bf),
            "mask": np.ascontiguousarray(
                np.asarray(mask[b], dtype=f32).reshape(S * T)
            ).astype(bf),
            "wq": Wq_r,
            "bq": bq_b,
            "wv": Wv_r,
            "wo": Wo_r,
            "bo2": bo2,
            "sel": sel_b,
            "grep": grep,
            "brep": brep,
            "ident": ident_b,
        }
        for b in range(N_CORES)
    ]
    res = run_bass_kernel_spmd(nc, in_maps, core_ids=list(range(N_CORES)))
    _NC_CACHE["last_results"] = res
    globals()["_LAST_RESULTS"] = res
    return np.stack([res.results[i]["out"] for i in range(N_CORES)], axis=0)
